# revision 4
# baseline (speedup 1.0000x reference)
"""Trainium2 Bass kernel for nn_DictConv2d (FISTA convolutional sparse coding).

Reference computation (per sample):
    Wn  = W / ||W||_F per output filter          (128, 64, 3, 3)
    c_1 = relu(MU*conv(x, Wn) - thr);  y_1 = c_1;  t_1 = 1
    repeat 5x:
        r       = x - conv_T(y_k, Wn)
        c_{k+1} = relu(y_k + MU*conv(r, Wn) - thr)
        y_{k+1} = (1+mu_k) c_{k+1} - mu_k c_k
    return c_6

u-form per iteration (b = MU*conv(x) - THR precomputed once, fp32):
    u = conv_T(y)   bf16 tap matmuls, col-tiled into two concurrent
                    tile_position groups (as v1); evicted fp8*SU
    c_{k+1} = relu(psum)/PSC * alpha with
        psum seeded by z*PSC = yf*PSC + b_s (one DVE STT into SBUF +
        one ACT copy into the bank; DVE writes PSUM 3x slow), then
        THREE fp8e4 DoubleRow matmuls (2 k-tiles each, 0.5 PE cyc/row,
        2x bf16 throughput) over the row-stacked fp8 upad:
            mm0 k-tiles {dx0 pair, dx2 pair}          stride 2
            mm1 k-tiles {dx1 pair, single (2,1)}      stride 2*PH
            mm2 k-tiles {single (2,0), single (2,2)}  stride 2
        DoubleRow k-tile strides must be >= 2 elements and DoubleRow
        cannot combine with tile_position (HW constraints, measured).
        The moving free axis is one contiguous NG*PH span including the
        pad columns; garbage outputs are never evicted.

Layout: every fwd-path tensor (c, yf, b) lives in PH-pitch rows (58
cols/row, 2 garbage cols) so all epilogue ops are fully CONTIGUOUS
[128,464] ops (strided SBUF/PSUM access is ~1.5x slower, strided
engine-writes to PSUM 3x). b_s garbage cols are poisoned to -1e30 once
at init => seeded psum garbage is hugely negative => relu writes c
garbage = 0 => momentum keeps yf garbage = 0 => the full-span ypad
copies spill zeros onto the padded-image borders, preserving the
border invariant with no strided ops.

conv_T eviction: 4 psum chunks -> one 58-pitch fp8 staging tile (ACT,
garbage cols zeroed once) -> 8 wide DMAs distribute into the stacked
upad (per-chunk DMAs cost ~600ns of SP issue time each; batching 4x
cut SP busy from 144us to 73us).  Output: last iteration evicts
compact via DVE and streams out in 4 contiguous DMAs per sample.

Engine budget per fwd chunk: PE 3 DR matmuls (585ns), ACT zcopy+2/3
cev (~1.1us), DVE zs + momentum + ypad + 1/3 cev (~1.9us across 2
samples).  Measured: PE 136us busy (77% occ), ACT 122us, DVE 122us.
213us (v1 bf16) -> 171us.

Numerics: fp8 u/weights add ~8e-3 on top of the bf16 dictionary floor
(2.4e-3): rel err 1.36e-2 vs the 2e-2 gate (z path stays fp32 exact).
Set FP8_FWD[k]=False to fall back to bf16 (6 taps) for iteration k.
"""

import math
import sys

sys.path.insert(0, "/opt/trn_rl_repo")

import numpy as np
import ml_dtypes

import concourse.bass as bass
import concourse.tile as tile
from concourse import mybir
from concourse import bass_utils
from concourse.ap import AP
from concourse.vector_clock import ScopedClock

F32 = mybir.dt.float32
BF16 = mybir.dt.bfloat16
FP8 = mybir.dt.float8e4
AF = mybir.ActivationFunctionType
ALU = mybir.AluOpType
DR = mybir.MatmulPerfMode.DoubleRow
E4 = ml_dtypes.float8_e4m3

MU = 0.1
THR = MU * 0.1
N_ITERS = 5
H = W_ = 56
PH = H + 2               # 58
NPAD = PH * PH           # 3364
NSLK = NPAD + PH         # + slack row for AP overreach
NPIX = H * W_            # 3136
NPP = H * PH             # 3248: PH-pitch fwd tensor size
NG = 8                   # output rows per fwd chunk
NCHUNK = H // NG         # 7
NFF = NG * PH            # 464 psum columns per fwd chunk
NCORES = 8
SPC = 2

SU = 32.0                # u fp8 scale
SWF = 8192.0             # fwd weight fp8 scale
PSC = SU * SWF
POISON = -1.0e30

FP8_FWD = (True, True, True, True, True)
SEED = "dve"             # "dve" | "act"

FWD_MMS = ((0 * PH + 0, 2), (0 * PH + 1, 2 * PH), (2 * PH + 0, 2))


def _fista_consts():
    t = 1.0
    mu = []
    for _ in range(N_ITERS):
        t_next = (1.0 + math.sqrt(1.0 + 4.0 * t * t)) / 2.0
        mu.append((t - 1.0) / t_next)
        t = t_next
    alpha = [mu[1], mu[2], mu[3], 1.0, 1.0]
    s = [None, (1.0 + mu[1]) / alpha[1], (1.0 + mu[2]) / alpha[2],
         (1.0 + mu[3]) / alpha[3], None]
    inv_a0 = 1.0 / alpha[0]
    return mu, alpha, s, inv_a0


# --------------------------------------------------------------------------
# Workarounds: walrus in this container rejects >1 sync-wait per NoOp and >4
# on other opcodes (see v1).
def _split_drain_and_barrier(self, tick_clock, wait_clock):
    nc = self.nc
    probe = nc.sync.nop()
    wait_clock.add_sem_waits(probe.ins, ScopedClock({None: tick_clock.global_clock}))
    ow = list(probe.ins.sync_info.on_wait) if probe.ins.sync_info else []
    probe.ins.sync_info = mybir.SyncInfo(on_wait=ow[:1], on_update=[])
    for w in ow[1:]:
        nop = nc.sync.nop()
        nop.ins.sync_info = mybir.SyncInfo(on_wait=[w], on_update=[])
    nc.sync.drain()
    nc.all_engine_barrier()
    assert self.sems is not None
    popped = nc._tile_sem_poison_stack.pop()
    assert popped is self._sem_poison
    nc.clear_and_free_semaphores(list(self.sems.allocated().values()))
    nc.all_engine_barrier()


tile.TileContext._drain_and_barrier = _split_drain_and_barrier

_WAIT_LIMIT = 1


def _hoist_excess_waits(nc):
    for fn in nc.m.functions:
        for blk in fn.blocks:
            insts = list(blk.instructions)
            out = []
            changed = False
            for inst in insts:
                si = inst.sync_info
                if si is not None and si.on_wait and len(si.on_wait) > _WAIT_LIMIT:
                    waits = list(si.on_wait)
                    keep = waits[-_WAIT_LIMIT:]
                    for w in waits[:-_WAIT_LIMIT]:
                        nop = mybir.InstNoOp(
                            name=nc.get_next_instruction_name(),
                            engine=inst.engine,
                            bass_nofuse=True,
                            sync_info=mybir.SyncInfo(on_wait=[w], on_update=[]),
                        )
                        nc.register_instruction(nop)
                        out.append(nop)
                    inst.sync_info = mybir.SyncInfo(
                        on_wait=keep, on_update=list(si.on_update)
                    )
                    changed = True
                out.append(inst)
            if changed:
                blk.instructions = out


# --------------------------------------------------------------------------
def _build_program():
    mu, alpha, s_k, inv_a0 = _fista_consts()

    nc = bass.Bass("TRN2", debug=False, num_devices=NCORES)

    for v in {-THR, 0.0, -PSC * THR}:
        t = nc.alloc_sbuf_tensor(f"const-f32-{v}", [128, 1], F32)
        nc.gpsimd.memset(t.ap(), v)
        nc.const_aps.aps[(F32, v)] = t.ap()
    nc.all_engine_barrier()

    xinb = nc.dram_tensor("xpadb", [SPC, 64, NSLK], BF16, kind="ExternalInput")
    wct_d = nc.dram_tensor("wct", [128, 9 * 64], BF16, kind="ExternalInput")
    wfp_d = nc.dram_tensor("wfp", [128, 3 * 128], BF16, kind="ExternalInput")
    wfs_d = nc.dram_tensor("wfs", [128, 3 * 128], BF16, kind="ExternalInput")
    wfn_d = nc.dram_tensor("wfn", [128, 6 * 128], BF16, kind="ExternalInput")
    wf8_d = nc.dram_tensor("wf8", [128, 3 * 2 * 128], FP8, kind="ExternalInput")
    out_d = nc.dram_tensor("out", [SPC, 128, NPIX], F32, kind="ExternalOutput")

    with tile.TileContext(nc) as tc:
        with (
            tc.tile_pool(name="pers", bufs=1) as pers,
            tc.tile_pool(name="psum", bufs=5, space="PSUM") as psum,
        ):
            wfp = pers.tile([128, 3 * 128], BF16, tag="wfp")
            wfs = pers.tile([128, 3 * 128], BF16, tag="wfs")
            wct = pers.tile([128, 9 * 64], BF16, tag="wct")
            wf8 = pers.tile([128, 3 * 2 * 128], FP8, tag="wf8")
            wfn = pers.tile([128, 6 * 128], BF16, tag="wfn")
            nc.sync.dma_start(out=wfp, in_=wfp_d.ap())
            nc.sync.dma_start(out=wfs, in_=wfs_d.ap())

            xsb, up8, ypad, yf, bf32, cbuf, cfin = [], [], [], [], [], [], []
            for s in range(SPC):
                xb = pers.tile([128, NSLK], BF16, tag=f"xb_{s}", name=f"xb_{s}")
                u8 = pers.tile([128, NSLK], FP8, tag=f"u8_{s}", name=f"u8_{s}")
                yp = pers.tile([128, NSLK], BF16, tag=f"yp_{s}", name=f"yp_{s}")
                yfs = pers.tile([128, NPP], F32, tag=f"yf_{s}", name=f"yf_{s}")
                bb = pers.tile([128, NPP], F32, tag=f"bb_{s}", name=f"bb_{s}")
                ca = pers.tile([128, NPP], F32, tag=f"ca_{s}", name=f"ca_{s}")
                cb = pers.tile([128, NPP], F32, tag=f"cb_{s}", name=f"cb_{s}")
                cf = pers.tile([128, NPIX], F32, tag=f"cf_{s}",
                               name=f"cf_{s}")
                xsb.append(xb); up8.append(u8); ypad.append(yp)
                yf.append(yfs); bf32.append(bb); cbuf.append((ca, cb))
                cfin.append(cf)
                # stacked bf16 x: partitions 0-63 = xpad, 64-127 one row on
                nc.gpsimd.memset(xb[:, NPAD:NSLK], 0.0)
                dma_q = nc.sync.dma_start if s == 0 else nc.scalar.dma_start
                for b0, b1 in ((0, 14), (14, 30), (30, 58)):
                    f0, f1 = b0 * PH, b1 * PH
                    dma_q(out=xb[0:64, f0:f1],
                          in_=xinb.ap()[s, :, f0:f1])
                    dma_q(out=xb[64:128, f0:f1],
                          in_=xinb.ap()[s, :, PH + f0:PH + f1])
                nc.gpsimd.memset(u8, 0.0)
                nc.gpsimd.memset(yp, 0.0)
            nc.scalar.dma_start(out=wf8, in_=wf8_d.ap())
            nc.scalar.dma_start(out=wct, in_=wct_d.ap())
            nc.scalar.dma_start(out=wfn, in_=wfn_d.ap())

            wf84 = wf8.rearrange("p (t two m) -> p t two m", t=3, two=2)

            u3 = [t.rearrange("p (r c) -> p r c", c=PH) for t in up8]
            y3 = [t.rearrange("p (r c) -> p r c", c=PH) for t in ypad]
            yp3 = [t.rearrange("p (r c) -> p r c", c=PH) for t in yf]
            bp3 = [t.rearrange("p (r c) -> p r c", c=PH) for t in bf32]

            def ch(t, g0):
                """contiguous PH-pitch chunk slice [128, NFF]"""
                return t[:, g0 * PH:g0 * PH + NFF]

            def fwd_fp8(s, g0, ptile):
                src = up8[s]
                pstr = src.ap[0][0]
                for i, (off0, stride) in enumerate(FWD_MMS):
                    rhs = AP(src.tensor, src.offset + g0 * PH + off0,
                             [[pstr, 128], [stride, 2], [1, NFF]])
                    nc.tensor.matmul(ptile, wf84[:, i], rhs,
                                     start=False, stop=(i == 2), perf_mode=DR)

            def fwd_bf16(s, g0, ptile, src_t, pair_w, sgl_w, seeded):
                src = src_t
                pstr = src.ap[0][0]
                first = not seeded
                for dx in range(3):
                    rhs = AP(src.tensor, src.offset + g0 * PH + dx,
                             [[pstr, 128], [1, NFF]])
                    nc.tensor.matmul(ptile, pair_w[:, dx * 128:(dx + 1) * 128],
                                     rhs, start=first, stop=False)
                    first = False
                for dx in range(3):
                    rhs = AP(src.tensor, src.offset + (g0 + 2) * PH + dx,
                             [[pstr, 128], [1, NFF]])
                    nc.tensor.matmul(ptile, sgl_w[:, dx * 128:(dx + 1) * 128],
                                     rhs, start=False, stop=(dx == 2))

            CT_CHUNKS = [(0, 8), (16, 8), (32, 8), (48, 4)]

            def convt_taps(src3, g0, h, pc):
                for t in range(9):
                    dy, dx = divmod(t, 3)
                    nc.tensor.matmul(
                        pc[0:64, :], wct[:, t * 64:(t + 1) * 64],
                        src3[:, g0 + dy:g0 + dy + h, dx:dx + W_],
                        start=(t == 0), stop=(t == 8), tile_position=(0, 0))
                    nc.tensor.matmul(
                        pc[64:128, :], wct[:, t * 64:(t + 1) * 64],
                        src3[:, g0 + h + dy:g0 + 2 * h + dy, dx:dx + W_],
                        start=(t == 0), stop=(t == 8), tile_position=(0, 64))

            # persistent 58-pitch staging (garbage cols memset-0 once; the
            # wide distribute DMAs then write zeros onto upad borders)
            stgf_t = []
            for s in range(SPC):
                sf = pers.tile([128, 4 * 8 * PH], FP8, tag=f"stgf_{s}",
                               name=f"stgf_{s}")
                nc.gpsimd.memset(sf, 0.0)
                stgf_t.append(sf)

            def emit_convt(s):
                """u = conv_T(y): 4 psum chunks evicted (fp8*SU) into one
                58-pitch staging tile, then 8 wide DMAs distribute into the
                stacked upad (per-chunk DMAs cost ~600ns SP issue each)."""
                stgf = stgf_t[s]
                sf3 = stgf.rearrange("p (g r c) -> p g r c", g=4, r=8)
                for ci, (g0, h) in enumerate(CT_CHUNKS):
                    pc = psum.tile([128, 8 * W_], F32, tag="pc", name="pc",
                                   bufs=3)
                    pcs = pc[:, 0:h * W_]
                    convt_taps(y3[s], g0, h, pcs)
                    nc.scalar.activation(
                        sf3[:, ci, 0:h, 0:W_],
                        pcs.rearrange("p (r c) -> p r c", c=W_),
                        AF.Copy, bias=0.0, scale=SU)
                ustr = up8[s].ap[0][0]
                uoff = up8[s].offset
                sstr = stgf.ap[0][0]
                soff = stgf.offset

                def dist(sp0, pdst, rbase, rtail):
                    src = AP(stgf.tensor, soff + sp0 * sstr,
                             [[sstr, 64], [8 * PH, 3], [1, 8 * PH]])
                    dst = AP(up8[s].tensor,
                             uoff + pdst * ustr + rbase * PH + 1,
                             [[ustr, 64], [16 * PH, 3], [1, 8 * PH]])
                    nc.sync.dma_start(out=dst, in_=src)
                    srct = AP(stgf.tensor, soff + sp0 * sstr + 3 * 8 * PH,
                              [[sstr, 64], [1, 4 * PH]])
                    dstt = AP(up8[s].tensor,
                              uoff + pdst * ustr + (48 + rtail) * PH + 1,
                              [[ustr, 64], [1, 4 * PH]])
                    nc.sync.dma_start(out=dstt, in_=srct)

                dist(0, 0, 1, 1)    # aligned lower -> rows g0+1
                dist(64, 64, 8, 4)  # aligned upper -> rows g0+h
                dist(0, 64, 0, 0)   # crossed upper -> rows g0
                dist(64, 0, 9, 5)   # crossed lower -> rows g0+1+h

            # ---- init: b_s = PSC*MU*conv(x) (PH-pitch); y_1 = relu(b-thr) --
            for s in range(SPC):
                for c in range(NCHUNK):
                    g0 = c * NG
                    pf = psum.tile([128, NFF], F32, tag="pf", name="pf")
                    fwd_bf16(s, g0, pf, xsb[s], wfp, wfs, False)
                    nc.scalar.activation(ch(bf32[s], g0), pf,
                                         AF.Copy, bias=-PSC * THR, scale=PSC)
                    nc.scalar.activation(ch(yf[s], g0), pf,
                                         AF.Relu, bias=-THR, scale=1.0)
                    # strided ypad copy (yf garbage cols not yet cleared)
                    nc.vector.tensor_copy(
                        y3[s][:, g0 + 1:g0 + 1 + NG, 1:1 + W_],
                        yp3[s][:, g0:g0 + NG, 0:W_])
                # poison b garbage cols; zero yf garbage cols
                nc.gpsimd.memset(bp3[s][:, :, W_:PH], POISON)
                nc.gpsimd.memset(yp3[s][:, :, W_:PH], 0.0)

            # ---- 5 FISTA iterations ---------------------------------------
            for k in range(N_ITERS):
                cdst = [cbuf[s][k % 2] for s in range(SPC)]
                csrc = [cbuf[s][(k + 1) % 2] for s in range(SPC)]
                last = k == N_ITERS - 1
                a = alpha[k]
                use8 = FP8_FWD[k]
                emit_convt(0)
                emit_convt(1)
                for s in range(SPC):
                    for c in range(NCHUNK):
                        c3 = cdst[s].rearrange("p (r c) -> p r c", c=PH)
                        g0 = c * NG
                        pf = psum.tile([128, NFF], F32, tag="pf", name="pf")
                        zs = pers.tile([128, NFF], F32, tag="zs",
                                       name="zs", bufs=4)
                        nc.vector.scalar_tensor_tensor(
                            zs, ch(yf[s], g0), PSC, ch(bf32[s], g0),
                            ALU.mult, ALU.add)
                        nc.scalar.activation(pf, zs, AF.Copy,
                                             bias=0.0, scale=1.0)
                        if use8:
                            fwd_fp8(s, g0, pf)
                        else:
                            fwd_bf16(s, g0, pf, up8[s], wfn[:, 0:3 * 128],
                                     wfn[:, 3 * 128:], True)
                        if last:
                            # evict compact (contiguous out-DMA at full BW),
                            # batch chunks per DMA to cut SP issue time
                            pfv = pf.rearrange("p (r c) -> p r c", c=PH)
                            cdst_v = cfin[s].rearrange(
                                "p (r c) -> p r c", c=W_)[:, g0:g0 + NG, :]
                            if c % 2 == 0:
                                nc.scalar.activation(cdst_v, pfv[:, :, 0:W_],
                                                     AF.Relu, bias=0.0,
                                                     scale=a / PSC)
                            else:
                                nc.vector.tensor_scalar(
                                    cdst_v, pfv[:, :, 0:W_], a / PSC, 0.0,
                                    ALU.mult, ALU.max)
                            if c in (1, 3, 5, 6):
                                lo = (c // 2) * 2 * NG * W_ if c < 6 \
                                    else 6 * NG * W_
                                fl = slice(lo, (c + 1) * NG * W_)
                                nc.sync.dma_start(
                                    out=out_d.ap()[s, :, fl],
                                    in_=cfin[s][:, fl])
                        else:
                            if c % 3 == 1:
                                nc.vector.tensor_scalar(
                                    ch(cdst[s], g0), pf, a / PSC, 0.0,
                                    ALU.mult, ALU.max)
                            else:
                                nc.scalar.activation(ch(cdst[s], g0), pf,
                                                     AF.Relu, bias=0.0,
                                                     scale=a / PSC)
                            if k == 0:
                                nc.vector.tensor_scalar_mul(
                                    ch(yf[s], g0), ch(cdst[s], g0), inv_a0)
                            else:
                                nc.vector.scalar_tensor_tensor(
                                    ch(yf[s], g0), ch(cdst[s], g0), s_k[k],
                                    ch(csrc[s], g0), ALU.mult, ALU.subtract)
                            nc.vector.tensor_copy(
                                ypad[s][:, g0 * PH + PH + 1:
                                        g0 * PH + PH + 1 + NFF],
                                ch(yf[s], g0))
    _hoist_excess_waits(nc)
    return nc


# --------------------------------------------------------------------------
def _host_prep(x, W):
    x = np.asarray(x, dtype=np.float32)
    W = np.asarray(W, dtype=np.float32)
    Wn = W / np.sqrt((W * W).sum(axis=(1, 2, 3), keepdims=True) + 1e-12)

    bf = ml_dtypes.bfloat16
    wct = np.empty((128, 9 * 64), dtype=np.float32)
    for t in range(9):
        dy, dx = divmod(t, 3)
        wct[:, t * 64:(t + 1) * 64] = Wn[:, :, 2 - dy, 2 - dx]
    wfp = np.empty((128, 3 * 128), dtype=np.float32)
    wfs = np.zeros((128, 3 * 128), dtype=np.float32)
    for dx in range(3):
        wfp[0:64, dx * 128:(dx + 1) * 128] = MU * Wn[:, :, 0, dx].T
        wfp[64:128, dx * 128:(dx + 1) * 128] = MU * Wn[:, :, 1, dx].T
        wfs[0:64, dx * 128:(dx + 1) * 128] = MU * Wn[:, :, 2, dx].T
    wfn = np.concatenate([-wfp, -wfs], axis=1)
    wf8 = np.zeros((128, 3, 2, 128), dtype=np.float32)
    FW = -MU * SWF
    for kt, dx in ((0, 0), (1, 2)):
        wf8[0:64, 0, kt, :] = FW * Wn[:, :, 0, dx].T
        wf8[64:128, 0, kt, :] = FW * Wn[:, :, 1, dx].T
    wf8[0:64, 1, 0, :] = FW * Wn[:, :, 0, 1].T
    wf8[64:128, 1, 0, :] = FW * Wn[:, :, 1, 1].T
    wf8[0:64, 1, 1, :] = FW * Wn[:, :, 2, 1].T
    wf8[0:64, 2, 0, :] = FW * Wn[:, :, 2, 0].T
    wf8[0:64, 2, 1, :] = FW * Wn[:, :, 2, 2].T

    n = x.shape[0]
    xpad = np.zeros((n, 64, PH, PH), dtype=np.float32)
    xpad[:, :, 1:1 + H, 1:1 + W_] = x
    xpad = xpad.reshape(n, 64, NPAD)
    xpad = np.concatenate(
        [xpad, np.zeros((n, 64, PH), dtype=np.float32)], axis=2)

    shared = {
        "wct": wct.astype(bf),
        "wfp": wfp.astype(bf),
        "wfs": wfs.astype(bf),
        "wfn": wfn.astype(bf),
        "wf8": wf8.reshape(128, -1).astype(E4),
    }
    xpadb = xpad.astype(bf)
    in_maps = []
    for core in range(NCORES):
        slb = xpadb[core * SPC:(core + 1) * SPC]
        in_maps.append({"xpadb": np.ascontiguousarray(slb), **shared})
    return in_maps


_CACHED_NC = None


def _get_nc():
    global _CACHED_NC
    if _CACHED_NC is None:
        _CACHED_NC = _build_program()
    return _CACHED_NC


def _run(x, W, **kwargs):
    in_maps = _host_prep(x, W)
    nc = _get_nc()
    res = bass_utils.run_bass_kernel_spmd(
        nc, in_maps, core_ids=list(range(NCORES)), **kwargs)
    outs = [res.results[i]["out"].reshape(SPC, 128, H, W_) for i in range(NCORES)]
    full = np.concatenate(outs, axis=0)
    return full, res


def kernel(x, W):
    out, _ = _run(x, W)
    return out


def kernel_profiled(x, W, tmpdir=None):
    _install_ntff_hook()
    out, res = _run(x, W, trace=True, tmpdir=tmpdir)
    return out, res


def _install_ntff_hook():
    import contextlib
    import ctypes
    import types

    try:
        from antenv.axon_hooks import get_axon_ntff_profile_hook  # noqa: F401
        return
    except ImportError:
        pass

    so_path = "/opt/axon/libaxon_pjrt.so"
    lib = ctypes.CDLL(so_path)
    if not hasattr(lib, "axon_start_nrt_profile"):
        return
    lib.axon_start_nrt_profile.argtypes = [
        ctypes.POINTER(ctypes.c_int64), ctypes.c_size_t]
    lib.axon_start_nrt_profile.restype = ctypes.c_int64
    lib.axon_stop_nrt_profile.argtypes = [ctypes.c_char_p]
    lib.axon_stop_nrt_profile.restype = ctypes.c_int64

    @contextlib.contextmanager
    def _hook(output_dir, device_ids):
        import jax
        jax.devices()
        if device_ids:
            ids = (ctypes.c_int64 * len(device_ids))(*device_ids)
            rc = lib.axon_start_nrt_profile(ids, len(device_ids))
        else:
            rc = lib.axon_start_nrt_profile(None, 0)
        if rc != 0:
            raise RuntimeError(f"axon_start_nrt_profile rc={rc}")
        try:
            yield
        finally:
            n = lib.axon_stop_nrt_profile(str(output_dir).encode())
            if n < 0:
                raise RuntimeError(f"axon_stop_nrt_profile rc={n}")
            if n == 0:
                print("WARNING: NTFF capture wrote no files")

    mod = types.ModuleType("antenv.axon_hooks")
    mod.get_axon_ntff_profile_hook = lambda: _hook
    mod.set_axon_ntff_profile_hook = lambda h: None
    sys.modules["antenv.axon_hooks"] = mod
